# revision 4
# baseline (speedup 1.0000x reference)
"""Trainium2 Bass kernel for nn_LoRALinear (out = x @ (W + s*L@R)^T + bias).

Full shapes: x [4, 2048, 4096], weight [4096, 4096], bias [4096],
lora_left [4096, 16], lora_right [16, 4096], out [4, 2048, 4096].

Sharding (8 cores, 2D): tokens split 4 ways (the batch dim) x d_out split
2 ways. Core i handles batch b = i % 4 and output half oh = i // 4, i.e. a
[2048, 2048] output block with the full K = 4096 contraction.

Host-side layout prep (part of sharding): the TensorEngine contracts over
the partition dim of both operands, so both x and W need d_in-major
layouts; fp32 has no DMA-transpose path on trn2. We pre-transpose the
shards on the host (np.ascontiguousarray) so the device kernel is pure
matmul. The LoRA right factor is pre-transposed too, and the scaled L^T is
shipped scaled; bias is shipped replicated to 128 partitions so the
kh=0 PSUM->SBUF copy doubles as the bias add, and the LoRA term joins the
kh=1 PSUM accumulation group as one extra 16-partition matmul.

Device blocking (per core, ~176KB/partition usable SBUF):
  loops kh in {0,1} (K halves of 2048) x tb in {0,1} (token halves of 1024);
  x^T[kh, tb] resident in SBUF ([128, 16, 1024] = 64KB/part, read once),
  w^T[kh] streamed in [128, 16, 512] o-chunks (32KB/part, double-buffered,
  read once per tb so twice overall);
  psum [128, 512] accumulates 16 matmuls; kh=0 writes partials to a DRAM
  scratch tile, kh=1 adds them back in.
"""

import os
import sys

import numpy as np

for _p in ("/root/.axon_site/_ro/trn_rl_repo", "/opt/trn_rl_repo"):
    if _p not in sys.path and os.path.isdir(_p):
        sys.path.append(_p)

import bass_rust
import concourse.bass as bass
import concourse.mybir as mybir
import concourse.tile as tile
from concourse.bass import ts
from concourse.bass_utils import run_bass_kernel_spmd
from concourse.vector_clock import ScopedClock, VectorClock

# ---- problem constants (hardcoded per contract) ----
B, S, D_IN, D_OUT, LORA_DIM = 4, 2048, 4096, 4096, 16
LORA_SCALE = 32.0 / LORA_DIM
N_CORES = 8
T = 2048          # tokens per core (= one batch element)
O = 2048          # d_out per core (half)
K = D_IN          # contraction
KH = K // 2       # K half resident in SBUF
NKT = KH // 128   # 16 k-tiles per half
TB = 1024         # token block
NTB = T // TB     # 2 token blocks
NTT_B = TB // 128  # 8 token tiles per block
OCW = 512         # o-chunk width (one PSUM bank)
NOC = O // OCW    # 4 o-chunks

# "f32r" (full-rate fp32, reduced-precision multiplies) or "f32" (exact fp32,
# 4 cycles/row). Flip via env for A/B testing; default chosen empirically.
COMPUTE = os.environ.get("LORA_KERNEL_COMPUTE", "f32r")
# dtype of the kh=0 partial written to DRAM scratch ("bf16" halves that
# round-trip's traffic; costs ~7e-4 rel err -- only the default when the
# matmul inputs are already bf16)
PARTIAL = os.environ.get(
    "LORA_KERNEL_PARTIAL", "bf16" if COMPUTE == "bf16" else "f32"
)

# Set by kernel() after a traced run (test.py reads it).
LAST_EXEC_TIME_NS = None
LAST_RESULT = None
TRACE = False


class SplitDrainTileContext(tile.TileContext):
    """TileContext that splits multi-wait instructions for this walrus build.

    This walrus rejects instructions carrying >2 sync waits ("Too many sync
    wait commands"). Engine queues are in-order, so an instruction's waits
    can equivalently ride same-engine NOPs inserted just before it; we cap
    every instruction at one wait. Same treatment for the exit Drain.
    """

    _splitw_counter = 0

    def _split_excess_waits(self, ordered):
        for bb_name, insts in ordered.items():
            new_list = []
            changed = False
            for inst in insts:
                si = getattr(inst, "sync_info", None)
                eng = getattr(inst, "engine", mybir.EngineType.Unassigned)
                waits = list(si.on_wait) if si is not None and si.on_wait else []
                if len(waits) > 1 and eng != mybir.EngineType.Unassigned:
                    # keep register-valued waits (if any) on the original
                    movable = [w for w in waits if w.wait_reg is None]
                    pinned = [w for w in waits if w.wait_reg is not None]
                    keep = pinned + movable[-1:] if not pinned else pinned
                    move = movable[:-1] if not pinned else movable
                    for w in move:
                        SplitDrainTileContext._splitw_counter += 1
                        nop = bass_rust.InstNoOp(
                            name=f"tile_splitw_{SplitDrainTileContext._splitw_counter}",
                            ins=[],
                            outs=[],
                        )
                        nop.engine = eng
                        nop.bass_nofuse = True
                        nop.sync_info = bass_rust.SyncInfo(
                            on_wait=[w], on_update=[]
                        )
                        new_list.append(nop)
                    inst.sync_info = bass_rust.SyncInfo(
                        on_wait=keep, on_update=list(si.on_update)
                    )
                    changed = True
                new_list.append(inst)
            if changed:
                insts[:] = new_list

    def _lower_ordered_insts(self, ordered):
        self._split_excess_waits(ordered)
        return super()._lower_ordered_insts(ordered)

    def _drain_and_barrier(self, tick_clock, wait_clock):
        g = tick_clock.global_clock
        for proc in range(len(g)):
            t = g[proc]
            if t <= 0:
                continue
            v = VectorClock()
            v.require_at_least(proc, t)
            nop = self.nc.sync.nop(nofuse=True)
            wait_clock.add_sem_waits(nop.ins, ScopedClock({None: v}))
        drain_inst = self.nc.sync.drain()
        wait_clock.add_sem_waits(
            drain_inst.ins, ScopedClock({None: g}), ScopedClock({None: g})
        )
        self.nc.all_engine_barrier()
        assert self.sems is not None
        popped = self.nc._tile_sem_poison_stack.pop()
        assert popped is self._sem_poison
        self.nc.clear_and_free_semaphores(list(self.sems.allocated().values()))
        self.nc.all_engine_barrier()


def _build_nc() -> bass.Bass:
    f32 = mybir.dt.float32
    mm_dt = {
        "f32r": mybir.dt.float32r,
        "bf16": mybir.dt.bfloat16,
        "f32": f32,
    }[COMPUTE]
    part_dt = mybir.dt.bfloat16 if PARTIAL == "bf16" else f32

    nc = bass.Bass("TRN2", target_bir_lowering=False, debug=False)
    # host-pre-tiled layouts: each SBUF tile's per-partition bytes are one
    # contiguous DRAM run (max-size DMA descriptors)
    xT = nc.declare_dram_parameter("xT", [2, NTB, 128, NKT, TB], mm_dt, isOutput=False)
    wT = nc.declare_dram_parameter("wT", [2, NOC, 128, NKT, OCW], mm_dt, isOutput=False)
    rT = nc.declare_dram_parameter("rT", [128, K // 128, LORA_DIM], mm_dt, isOutput=False)
    lT = nc.declare_dram_parameter("lT", [LORA_DIM, O], f32, isOutput=False)
    biasr = nc.declare_dram_parameter("biasr", [128, O], f32, isOutput=False)
    out = nc.declare_dram_parameter("out", [T, O], f32, isOutput=True)

    with SplitDrainTileContext(nc) as tc:
        with (
            tc.tile_pool(name="xt", bufs=1) as xt_pool,
            tc.tile_pool(name="wt", bufs=2) as wt_pool,
            tc.tile_pool(name="consts", bufs=1) as const_pool,
            tc.tile_pool(name="outsb", bufs=3) as out_pool,
            tc.tile_pool(name="partsb", bufs=3) as part_pool,
            tc.tile_pool(name="bias", bufs=2) as bias_pool,
            tc.tile_pool(name="psum", bufs=6, space="PSUM") as psum_pool,
            tc.tile_pool(name="psum1", bufs=2, space="PSUM") as psum1_pool,
            tc.tile_pool(name="dram", bufs=1, space="DRAM") as dram_pool,
        ):
            # constants: R^T (full K), [s*L^T; bias], xR^T-plus-ones row
            rt_sb = const_pool.tile([128, K // 128, LORA_DIM], mm_dt)
            nc.sync.dma_start(rt_sb[:], rT[:])
            lt_sb = const_pool.tile([LORA_DIM, O], f32)
            nc.sync.dma_start(lt_sb[:], lT[:])
            xr = const_pool.tile([LORA_DIM, T], f32)
            bias_tiles = {}

            # PE warm-up: dependency-free matmuls on garbage SBUF run while
            # the first x/W loads are in flight, so the HAM clock gate is at
            # 8/8 (2.4 GHz) when real matmuls start and the PE never idles
            # through the initial DMA window. Results are never read.
            warm = const_pool.tile([128, 512], f32)
            nc.any.memset(warm[:], 0.0)
            for _ in range(24):
                pw = psum1_pool.tile([128, 512], f32, tag="p1")
                nc.tensor.matmul(
                    pw[:], warm[:, :128], warm[:], start=True, stop=True
                )

            partial = dram_pool.tile([T, O], part_dt)

            for kh in range(2):
                for tb in range(NTB):
                    # token-split halves with separate tags: the next block's
                    # A-half load overlaps matmuls still reading this block's
                    # B-half (free double-buffering at no extra SBUF)
                    xtA = xt_pool.tile([128, NKT, TB // 2], mm_dt, tag="xtA")
                    xtB = xt_pool.tile([128, NKT, TB // 2], mm_dt, tag="xtB")
                    x_src = xT[kh, tb]
                    for q in range(2):
                        nq = NKT // 2
                        nc.sync.dma_start(
                            xtA[:, q * nq : (q + 1) * nq, :],
                            x_src[:, q * nq : (q + 1) * nq, : TB // 2],
                        )
                        nc.sync.dma_start(
                            xtB[:, q * nq : (q + 1) * nq, :],
                            x_src[:, q * nq : (q + 1) * nq, TB // 2 :],
                        )

                    # stage A: xR^T[j, t] accumulated over kh into xr rows 0..15
                    for c in range(TB // 512):
                        tg = tb * (TB // 512) + c
                        xth = xtA if c == 0 else xtB
                        p1 = psum1_pool.tile([LORA_DIM, 512], f32, tag="p1")
                        for k in range(NKT):
                            nc.tensor.matmul(
                                p1[:],
                                rt_sb[:, kh * NKT + k, :],
                                xth[:, k, :],
                                start=(k == 0),
                                stop=(k == NKT - 1),
                            )
                        if kh == 0:
                            nc.vector.tensor_copy(xr[:, ts(tg, 512)], p1[:])
                        else:
                            nc.vector.tensor_add(
                                xr[:, ts(tg, 512)], xr[:, ts(tg, 512)], p1[:]
                            )

                    # main: psum[t-tile, o-chunk] over this K half
                    for oc in range(NOC):
                        wt = wt_pool.tile([128, NKT, OCW], mm_dt, tag="wt")
                        nc.sync.dma_start(wt[:], wT[kh, oc])
                        for tt in range(NTT_B):
                            gt = tb * NTT_B + tt  # global token tile
                            xth = xtA if tt < NTT_B // 2 else xtB
                            th = tt % (NTT_B // 2)
                            ps = psum_pool.tile([128, OCW], f32, tag="ps")
                            for k in range(NKT):
                                nc.tensor.matmul(
                                    ps[:],
                                    xth[:, k, ts(th, 128)],
                                    wt[:, k, :],
                                    start=(k == 0),
                                    stop=(kh == 0 and k == NKT - 1),
                                )
                            if kh == 0:
                                # bias-add rides the psum->SBUF copy
                                if tt == 0:
                                    bias_sb = bias_pool.tile(
                                        [128, OCW], f32, tag="bias"
                                    )
                                    nc.sync.dma_start(
                                        bias_sb[:], biasr[:, ts(oc, OCW)]
                                    )
                                    bias_tiles[oc] = bias_sb
                                ob = out_pool.tile([128, OCW], part_dt, tag="ob")
                                nc.vector.tensor_add(
                                    ob[:], ps[:], bias_tiles[oc][:]
                                )
                                nc.sync.dma_start(
                                    partial[ts(gt, 128), ts(oc, OCW)], ob[:]
                                )
                            else:
                                # LoRA + bias ride the same accumulation group
                                # (kept exact fp32: tiny op count).
                                nc.tensor.matmul(
                                    ps[:],
                                    xr[:, ts(gt, 128)],
                                    lt_sb[:, ts(oc, OCW)],
                                    start=False,
                                    stop=True,
                                )
                                pb = part_pool.tile([128, OCW], part_dt, tag="pb")
                                nc.sync.dma_start(
                                    pb[:], partial[ts(gt, 128), ts(oc, OCW)]
                                )
                                ob = out_pool.tile([128, OCW], f32, tag="ob")
                                nc.vector.tensor_add(ob[:], ps[:], pb[:])
                                nc.sync.dma_start(
                                    out[ts(gt, 128), ts(oc, OCW)], ob[:]
                                )
    return nc


def kernel(**inputs: np.ndarray) -> np.ndarray:
    global LAST_EXEC_TIME_NS, LAST_RESULT

    x = np.ascontiguousarray(np.asarray(inputs["x"], dtype=np.float32))
    weight = np.asarray(inputs["weight"], dtype=np.float32)
    bias = np.asarray(inputs["bias"], dtype=np.float32)
    lora_left = np.asarray(inputs["lora_left"], dtype=np.float32)
    lora_right = np.asarray(inputs["lora_right"], dtype=np.float32)

    # host-side shard + layout prep (tiled to match SBUF tile order)
    # xT[kh, tb, p, ko, t'] = x[b][tb*TB + t', kh*KH + ko*128 + p]
    xT_shards = [
        np.ascontiguousarray(
            x[b].T.reshape(2, NKT, 128, NTB, TB).transpose(0, 3, 2, 1, 4)
        )
        for b in range(B)
    ]
    # wT[kh, oc, p, ko, o'] = weight[oh*O + oc*OCW + o', kh*KH + ko*128 + p]
    wT_halves = [
        np.ascontiguousarray(
            weight[oh * O : (oh + 1) * O, :].T
            .reshape(2, NKT, 128, NOC, OCW)
            .transpose(0, 3, 2, 1, 4)
        )
        for oh in range(2)
    ]
    # rT[p, ko, j] = lora_right[j, ko*128 + p]
    rT = np.ascontiguousarray(lora_right.T.reshape(K // 128, 128, LORA_DIM).transpose(1, 0, 2))
    lT_halves = [
        np.ascontiguousarray(LORA_SCALE * lora_left[oh * O : (oh + 1) * O, :].T)
        for oh in range(2)
    ]
    bias_halves = [
        np.ascontiguousarray(
            np.broadcast_to(bias[None, oh * O : (oh + 1) * O], (128, O))
        )
        for oh in range(2)
    ]

    if COMPUTE == "bf16":
        import ml_dtypes

        bf16 = ml_dtypes.bfloat16
        xT_shards = [s.astype(bf16) for s in xT_shards]
        wT_halves = [s.astype(bf16) for s in wT_halves]
        rT = rT.astype(bf16)

    in_maps = []
    for i in range(N_CORES):
        b, oh = i % B, i // B
        in_maps.append(
            {
                "xT": xT_shards[b],
                "wT": wT_halves[oh],
                "rT": rT,
                "lT": lT_halves[oh],
                "biasr": bias_halves[oh],
            }
        )

    nc = _build_nc()
    res = run_bass_kernel_spmd(
        nc, in_maps, core_ids=list(range(N_CORES)), trace=TRACE
    )
    LAST_EXEC_TIME_NS = res.exec_time_ns
    LAST_RESULT = res

    out = np.empty((B, S, D_OUT), dtype=np.float32)
    for i in range(N_CORES):
        b, oh = i % B, i // B
        out[b, :, oh * O : (oh + 1) * O] = res.results[i]["out"]
    return out



# revision 5
# speedup vs baseline: 1.1118x; 1.1118x over previous
"""Trainium2 Bass kernel for nn_LoRALinear (out = x @ (W + s*L@R)^T + bias).

Full shapes: x [4, 2048, 4096], weight [4096, 4096], bias [4096],
lora_left [4096, 16], lora_right [16, 4096], out [4, 2048, 4096].

Sharding (8 cores, 2D): tokens split 4 ways (the batch dim) x d_out split
2 ways. Core i handles batch b = i % 4 and output half oh = i // 4, i.e. a
[2048, 2048] output block with the full K = 4096 contraction.

Design (v2): single-pass full-K PSUM accumulation with a resident merged
weight. The LoRA factors are folded into W on-device at startup:
W_eff^T[k, o] = W^T[k, o] + (R^T (s L)^T)[k, o], computed as one
16-contraction matmul per [128 k x 512 o] tile with the sum riding an
in-place vector add into the freshly-DMA'd W tile. W_eff then stays
resident in SBUF in bf16 (128 KB/partition), x streams through in bf16
(16 KB/partition per 256-token block, double buffered), and each
[128 token x 512 out] PSUM tile accumulates all 32 k-tiles in one group
-- no DRAM partial round-trip (the v1 kernel's kh split cost 64 MiB of
extra HBM traffic and ~100 us). bf16 operands also keep the PE at its
1 cycle/row peak where the v1 f32r kernel measured a 227 ns steady-state
cadence per 512-row matmul (LDWEIGHTS of an fp32 stationary is ~4x the
bf16 cost and was not fully hidden).

Per-core totals: 2048 main matmuls (437 us of PE at 2.4 GHz) + 128 merge
matmuls (27 us, overlapping the W DMA window) + ~51 MiB of DMA (~150 us,
3x under the PE time). Bias rides the PSUM->SBUF copy as a vector add
against a 128-partition-replicated bias tile.

DMA ordering: all input DMAs (W quads, x block 0, bias) are enqueued
before any output DMA, and each x block is prefetched one iteration
ahead of use, so in-order queue processing never stalls an input load
behind an output store's semaphore wait.
"""

import os
import sys

import numpy as np

for _p in ("/root/.axon_site/_ro/trn_rl_repo", "/opt/trn_rl_repo"):
    if _p not in sys.path and os.path.isdir(_p):
        sys.path.append(_p)

import bass_rust
import concourse.bass as bass
import concourse.mybir as mybir
import concourse.tile as tile
from concourse.bass import ts
from concourse.bass_utils import run_bass_kernel_spmd
from concourse.vector_clock import ScopedClock, VectorClock

# ---- problem constants (hardcoded per contract) ----
B, S, D_IN, D_OUT, LORA_DIM = 4, 2048, 4096, 4096, 16
LORA_SCALE = 32.0 / LORA_DIM
N_CORES = 8
T = 2048          # tokens per core (= one batch element)
O = 2048          # d_out per core (half)
K = D_IN          # contraction
NKT = K // 128    # 32 k-tiles
TB = 256          # token block streamed per x DMA
NTB = T // TB     # 8 token blocks
NTT_B = TB // 128  # 2 token tiles per block
OCW = 512         # o-chunk width (one PSUM bank)
NOC = O // OCW    # 4 o-chunks
NWQ = 4           # W DMA quads per o-chunk (8 k-tiles each)
N_WARMUP = 20     # PE p-state warmup matmuls

# Set by kernel() after a traced run (test.py reads it).
LAST_EXEC_TIME_NS = None
LAST_RESULT = None
TRACE = False
COMPUTE = "bf16"


class SplitDrainTileContext(tile.TileContext):
    """TileContext that splits multi-wait instructions for this walrus build.

    This walrus rejects instructions carrying >2 sync waits ("Too many sync
    wait commands"). Engine queues are in-order, so an instruction's waits
    can equivalently ride same-engine NOPs inserted just before it; we cap
    every instruction at one wait. Same treatment for the exit Drain.
    """

    _splitw_counter = 0

    def _split_excess_waits(self, ordered):
        for bb_name, insts in ordered.items():
            new_list = []
            changed = False
            for inst in insts:
                si = getattr(inst, "sync_info", None)
                eng = getattr(inst, "engine", mybir.EngineType.Unassigned)
                waits = list(si.on_wait) if si is not None and si.on_wait else []
                if len(waits) > 1 and eng != mybir.EngineType.Unassigned:
                    # keep register-valued waits (if any) on the original
                    movable = [w for w in waits if w.wait_reg is None]
                    pinned = [w for w in waits if w.wait_reg is not None]
                    keep = pinned + movable[-1:] if not pinned else pinned
                    move = movable[:-1] if not pinned else movable
                    for w in move:
                        SplitDrainTileContext._splitw_counter += 1
                        nop = bass_rust.InstNoOp(
                            name=f"tile_splitw_{SplitDrainTileContext._splitw_counter}",
                            ins=[],
                            outs=[],
                        )
                        nop.engine = eng
                        nop.bass_nofuse = True
                        nop.sync_info = bass_rust.SyncInfo(
                            on_wait=[w], on_update=[]
                        )
                        new_list.append(nop)
                    inst.sync_info = bass_rust.SyncInfo(
                        on_wait=keep, on_update=list(si.on_update)
                    )
                    changed = True
                new_list.append(inst)
            if changed:
                insts[:] = new_list

    def _lower_ordered_insts(self, ordered):
        self._split_excess_waits(ordered)
        return super()._lower_ordered_insts(ordered)

    def _drain_and_barrier(self, tick_clock, wait_clock):
        g = tick_clock.global_clock
        for proc in range(len(g)):
            t = g[proc]
            if t <= 0:
                continue
            v = VectorClock()
            v.require_at_least(proc, t)
            nop = self.nc.sync.nop(nofuse=True)
            wait_clock.add_sem_waits(nop.ins, ScopedClock({None: v}))
        drain_inst = self.nc.sync.drain()
        wait_clock.add_sem_waits(
            drain_inst.ins, ScopedClock({None: g}), ScopedClock({None: g})
        )
        self.nc.all_engine_barrier()
        assert self.sems is not None
        popped = self.nc._tile_sem_poison_stack.pop()
        assert popped is self._sem_poison
        self.nc.clear_and_free_semaphores(list(self.sems.allocated().values()))
        self.nc.all_engine_barrier()


def _build_nc() -> bass.Bass:
    f32 = mybir.dt.float32
    bf = mybir.dt.bfloat16

    nc = bass.Bass("TRN2", target_bir_lowering=False, debug=False)
    # host-pre-tiled layouts: each SBUF tile's per-partition bytes are one
    # contiguous DRAM run (max-size DMA descriptors)
    xT = nc.declare_dram_parameter("xT", [NTB, 128, NKT, TB], bf, isOutput=False)
    wT = nc.declare_dram_parameter("wT", [128, NOC, NKT, OCW], bf, isOutput=False)
    rT = nc.declare_dram_parameter("rT", [LORA_DIM, K], bf, isOutput=False)
    lT = nc.declare_dram_parameter("lT", [LORA_DIM, O], bf, isOutput=False)
    biasr = nc.declare_dram_parameter("biasr", [128, O], f32, isOutput=False)
    out = nc.declare_dram_parameter("out", [T, O], f32, isOutput=True)

    with SplitDrainTileContext(nc) as tc:
        with (
            tc.tile_pool(name="consts", bufs=1) as const_pool,
            tc.tile_pool(name="xt", bufs=2) as xt_pool,
            tc.tile_pool(name="outsb", bufs=3) as out_pool,
            tc.tile_pool(name="psum", bufs=6, space="PSUM") as psum_pool,
            tc.tile_pool(name="psum_m", bufs=2, space="PSUM") as psum_m_pool,
        ):
            # small constants first: R^T, s*L^T (the merge matmul operands)
            rt_sb = const_pool.tile([LORA_DIM, K], bf)
            nc.sync.dma_start(rt_sb[:], rT[:])
            lt_sb = const_pool.tile([LORA_DIM, O], bf)
            nc.sync.dma_start(lt_sb[:], lT[:])

            # resident merged weight: [128 kpart, oc, kt, o'] bf16
            weff = const_pool.tile([128, NOC, NKT, OCW], bf)
            xt_tiles = {}

            def load_x(tb):
                xt = xt_pool.tile([128, NKT, TB], bf, tag="xt")
                nc.sync.dma_start(xt[:], xT[tb])
                xt_tiles[tb] = xt

            # input DMA order: W oc0 -> x tb0 -> bias -> W oc1..3 (all ahead
            # of any output DMA in the queue)
            nkq = NKT // NWQ
            for q in range(NWQ):
                nc.sync.dma_start(
                    weff[:, 0, q * nkq : (q + 1) * nkq, :],
                    wT[:, 0, q * nkq : (q + 1) * nkq, :],
                )
            load_x(0)
            bias_sb = const_pool.tile([128, O], f32)
            nc.sync.dma_start(bias_sb[:], biasr[:])
            for oc in range(1, NOC):
                for q in range(NWQ):
                    nc.sync.dma_start(
                        weff[:, oc, q * nkq : (q + 1) * nkq, :],
                        wT[:, oc, q * nkq : (q + 1) * nkq, :],
                    )

            # PE warm-up: dependency-free matmuls on garbage SBUF run while
            # the first W/x loads are in flight, so the HAM clock gate is at
            # 8/8 (2.4 GHz) when real matmuls start. Results are never read.
            warm = const_pool.tile([128, OCW], bf)
            nc.any.memset(warm[:], 0.0)
            for _ in range(N_WARMUP):
                pw = psum_m_pool.tile([128, OCW], f32, tag="pm")
                nc.tensor.matmul(
                    pw[:], warm[:, :128], warm[:], start=True, stop=True
                )

            def merge(oc):
                # weff[:, oc, kt, :] += (R^T (sL)^T)[k-tile kt, o-chunk oc]
                for kt in range(NKT):
                    pm = psum_m_pool.tile([128, OCW], f32, tag="pm")
                    nc.tensor.matmul(
                        pm[:],
                        rt_sb[:, ts(kt, 128)],
                        lt_sb[:, ts(oc, OCW)],
                        start=True,
                        stop=True,
                    )
                    nc.vector.tensor_add(
                        weff[:, oc, kt, :], weff[:, oc, kt, :], pm[:]
                    )

            def group(tb, tt, oc):
                xt = xt_tiles[tb]
                ps = psum_pool.tile([128, OCW], f32, tag="ps")
                for kt in range(NKT):
                    nc.tensor.matmul(
                        ps[:],
                        xt[:, kt, ts(tt, 128)],
                        weff[:, oc, kt, :],
                        start=(kt == 0),
                        stop=(kt == NKT - 1),
                    )
                # bias-add rides the psum->SBUF copy
                ob = out_pool.tile([128, OCW], f32, tag="ob")
                nc.vector.tensor_add(ob[:], ps[:], bias_sb[:, ts(oc, OCW)])
                nc.sync.dma_start(
                    out[ts(tb * NTT_B + tt, 128), ts(oc, OCW)], ob[:]
                )

            # first token block interleaved with the per-oc merges so the PE
            # never waits on the full W load
            for oc in range(NOC):
                merge(oc)
                for tt in range(NTT_B):
                    group(0, tt, oc)

            for tb in range(1, NTB):
                load_x(tb)  # prefetch ahead of this iteration's out DMAs
                for oc in range(NOC):
                    for tt in range(NTT_B):
                        group(tb, tt, oc)
    return nc


def kernel(**inputs: np.ndarray) -> np.ndarray:
    global LAST_EXEC_TIME_NS, LAST_RESULT
    import ml_dtypes

    bf16 = ml_dtypes.bfloat16

    x = np.asarray(inputs["x"], dtype=np.float32)
    weight = np.asarray(inputs["weight"], dtype=np.float32)
    bias = np.asarray(inputs["bias"], dtype=np.float32)
    lora_left = np.asarray(inputs["lora_left"], dtype=np.float32)
    lora_right = np.asarray(inputs["lora_right"], dtype=np.float32)

    # host-side shard + layout prep (tiled to match SBUF tile order)
    # xT[tb, p, kt, t'] = x[b][tb*TB + t', kt*128 + p]
    xT_shards = [
        np.ascontiguousarray(
            x[b].T.reshape(NKT, 128, NTB, TB).transpose(2, 1, 0, 3)
        ).astype(bf16)
        for b in range(B)
    ]
    # wT[p, oc, kt, o'] = weight[oh*O + oc*OCW + o', kt*128 + p]
    wT_halves = [
        np.ascontiguousarray(
            weight[oh * O : (oh + 1) * O, :].T
            .reshape(NKT, 128, NOC, OCW)
            .transpose(1, 2, 0, 3)
        ).astype(bf16)
        for oh in range(2)
    ]
    rT = np.ascontiguousarray(lora_right).astype(bf16)
    lT_halves = [
        np.ascontiguousarray(LORA_SCALE * lora_left[oh * O : (oh + 1) * O, :].T).astype(
            bf16
        )
        for oh in range(2)
    ]
    bias_halves = [
        np.ascontiguousarray(
            np.broadcast_to(bias[None, oh * O : (oh + 1) * O], (128, O))
        )
        for oh in range(2)
    ]

    in_maps = []
    for i in range(N_CORES):
        b, oh = i % B, i // B
        in_maps.append(
            {
                "xT": xT_shards[b],
                "wT": wT_halves[oh],
                "rT": rT,
                "lT": lT_halves[oh],
                "biasr": bias_halves[oh],
            }
        )

    nc = _build_nc()
    res = run_bass_kernel_spmd(
        nc, in_maps, core_ids=list(range(N_CORES)), trace=TRACE
    )
    LAST_EXEC_TIME_NS = res.exec_time_ns
    LAST_RESULT = res

    out = np.empty((B, S, D_OUT), dtype=np.float32)
    for i in range(N_CORES):
        b, oh = i % B, i // B
        out[b, :, oh * O : (oh + 1) * O] = res.results[i]["out"]
    return out


# revision 13
# speedup vs baseline: 1.1863x; 1.0670x over previous
"""Trainium2 Bass kernel for nn_LoRALinear (out = x @ (W + s*L@R)^T + bias).

Full shapes: x [4, 2048, 4096], weight [4096, 4096], bias [4096],
lora_left [4096, 16], lora_right [16, 4096], out [4, 2048, 4096].

Sharding (8 cores, 2D): tokens split 4 ways (the batch dim) x d_out split
2 ways. Core i handles batch b = i % 4 and output half oh = i // 4, i.e. a
[2048, 2048] output block with the full K = 4096 contraction.

Design (v3): single-pass full-K PSUM accumulation with resident raw W.
W stays resident in SBUF in bf16 (128 KB/partition), x streams through
in bf16 (16 KB/partition per 256-token block, double buffered), and each
[128 token x 512 out] PSUM tile accumulates all 32 k-tiles in one group
-- no DRAM partial round-trip (the v1 kernel's kh split cost 64 MiB of
extra HBM traffic and ~100 us). bf16 operands also keep the PE at its
1 cycle/row peak where the v1 f32r kernel measured a 227 ns steady-state
cadence per 512-row matmul (LDWEIGHTS of an fp32 stationary is ~4x the
bf16 cost and was not fully hidden).

LoRA path: per token block, xr = R @ x^T is one 32-matmul PSUM group
([16, 256], full-K accumulation) copied to SBUF bf16; each main PSUM
group then takes a 33rd matmul (xr^T-slice x sL^T-chunk) before stop.
A v2 variant instead pre-merged s*L@R into the resident W with 128
in-place vector adds at startup; that backlogged the vector engine for
~90 us, and the resulting PSUM-reuse stalls on the PE held the HAM
clock gate at 4/8 for ~120 us (569 us total). Keeping the LoRA term on
the PE (+14 us) and the vector engine nearly idle is the faster trade.

Per-core totals: 2048 main + 64 joint + 256 xr matmuls (~478 us of PE
at 2.4 GHz) + ~51 MiB of DMA (~150 us, 3x under the PE time). Bias
rides the PSUM->SBUF copy as a vector add against a
128-partition-replicated bias tile.

DMA ordering: all input DMAs (x block 0, W o-chunks, bias) are enqueued
before any output DMA, and each x block is prefetched one iteration
ahead of use, so in-order queue processing never stalls an input load
behind an output store's semaphore wait. PE warm-up matmuls bracket the
xr stage so the PE has no >700 ns idle gap before steady state (idle
gaps drop the HAM clock gate, which takes ~10 us of busy to re-ramp).
"""

import os
import sys

import numpy as np

for _p in ("/root/.axon_site/_ro/trn_rl_repo", "/opt/trn_rl_repo"):
    if _p not in sys.path and os.path.isdir(_p):
        sys.path.append(_p)

import bass_rust
import concourse.bass as bass
import concourse.mybir as mybir
import concourse.tile as tile
from concourse.bass import ts
from concourse.bass_utils import run_bass_kernel_spmd
from concourse.vector_clock import ScopedClock, VectorClock

# ---- problem constants (hardcoded per contract) ----
B, S, D_IN, D_OUT, LORA_DIM = 4, 2048, 4096, 4096, 16
LORA_SCALE = 32.0 / LORA_DIM
N_CORES = 8
T = 2048          # tokens per core (= one batch element)
O = 2048          # d_out per core (half)
K = D_IN          # contraction
NKT = K // 128    # 32 k-tiles
TB = 256          # token block streamed per x DMA
NTB = T // TB     # 8 token blocks
NTT_B = TB // 128  # 2 token tiles per block
OCW = 512         # o-chunk width (one PSUM bank)
NOC = O // OCW    # 4 o-chunks
N_WARMUP_PRE = 16   # PE warmups before the first xr stage (x block 0 DMA window)
N_WARMUP_POST = 18  # PE warmups between xr and the first main group (W oc0 window)

# Set by kernel() after a traced run (test.py reads it).
LAST_EXEC_TIME_NS = None
LAST_RESULT = None
TRACE = False
COMPUTE = "bf16"


class SplitDrainTileContext(tile.TileContext):
    """TileContext that splits multi-wait instructions for this walrus build.

    This walrus rejects instructions carrying >2 sync waits ("Too many sync
    wait commands"). Engine queues are in-order, so an instruction's waits
    can equivalently ride same-engine NOPs inserted just before it; we cap
    every instruction at one wait. Same treatment for the exit Drain.
    """

    _splitw_counter = 0

    def _split_excess_waits(self, ordered):
        for bb_name, insts in ordered.items():
            new_list = []
            changed = False
            for inst in insts:
                si = getattr(inst, "sync_info", None)
                eng = getattr(inst, "engine", mybir.EngineType.Unassigned)
                waits = list(si.on_wait) if si is not None and si.on_wait else []
                if len(waits) > 1 and eng != mybir.EngineType.Unassigned:
                    # keep register-valued waits (if any) on the original
                    movable = [w for w in waits if w.wait_reg is None]
                    pinned = [w for w in waits if w.wait_reg is not None]
                    keep = pinned + movable[-1:] if not pinned else pinned
                    move = movable[:-1] if not pinned else movable
                    for w in move:
                        SplitDrainTileContext._splitw_counter += 1
                        nop = bass_rust.InstNoOp(
                            name=f"tile_splitw_{SplitDrainTileContext._splitw_counter}",
                            ins=[],
                            outs=[],
                        )
                        nop.engine = eng
                        nop.bass_nofuse = True
                        nop.sync_info = bass_rust.SyncInfo(
                            on_wait=[w], on_update=[]
                        )
                        new_list.append(nop)
                    inst.sync_info = bass_rust.SyncInfo(
                        on_wait=keep, on_update=list(si.on_update)
                    )
                    changed = True
                new_list.append(inst)
            if changed:
                insts[:] = new_list

    def _lower_ordered_insts(self, ordered):
        self._split_excess_waits(ordered)
        return super()._lower_ordered_insts(ordered)

    def _drain_and_barrier(self, tick_clock, wait_clock):
        g = tick_clock.global_clock
        for proc in range(len(g)):
            t = g[proc]
            if t <= 0:
                continue
            v = VectorClock()
            v.require_at_least(proc, t)
            nop = self.nc.sync.nop(nofuse=True)
            wait_clock.add_sem_waits(nop.ins, ScopedClock({None: v}))
        drain_inst = self.nc.sync.drain()
        wait_clock.add_sem_waits(
            drain_inst.ins, ScopedClock({None: g}), ScopedClock({None: g})
        )
        self.nc.all_engine_barrier()
        assert self.sems is not None
        popped = self.nc._tile_sem_poison_stack.pop()
        assert popped is self._sem_poison
        self.nc.clear_and_free_semaphores(list(self.sems.allocated().values()))
        self.nc.all_engine_barrier()


def _build_nc() -> bass.Bass:
    f32 = mybir.dt.float32
    bf = mybir.dt.bfloat16

    nc = bass.Bass("TRN2", target_bir_lowering=False, debug=False)
    # host-pre-tiled layouts: each SBUF tile's per-partition bytes are one
    # contiguous DRAM run (max-size DMA descriptors)
    xT = nc.declare_dram_parameter("xT", [NTB, 128, NKT, TB], bf, isOutput=False)
    wT = nc.declare_dram_parameter("wT", [128, NOC, NKT, OCW], bf, isOutput=False)
    rT = nc.declare_dram_parameter("rT", [128, NKT, LORA_DIM], bf, isOutput=False)
    lT = nc.declare_dram_parameter("lT", [LORA_DIM, O], bf, isOutput=False)
    biasr = nc.declare_dram_parameter("biasr", [128, O], f32, isOutput=False)
    out = nc.declare_dram_parameter("out", [T, O], f32, isOutput=True)

    with SplitDrainTileContext(nc) as tc:
        with (
            tc.tile_pool(name="consts", bufs=1) as const_pool,
            tc.tile_pool(name="xt", bufs=2) as xt_pool,
            tc.tile_pool(name="xr", bufs=2) as xr_pool,
            tc.tile_pool(name="outsb", bufs=3) as out_pool,
            tc.tile_pool(name="psum", bufs=6, space="PSUM") as psum_pool,
            tc.tile_pool(name="psum_w", bufs=1, space="PSUM") as psum_w_pool,
            tc.tile_pool(name="psum_r", bufs=1, space="PSUM") as psum_r_pool,
        ):
            # small constants first: R^T, s*L^T (the LoRA matmul operands)
            rt_sb = const_pool.tile([128, NKT, LORA_DIM], bf)
            nc.sync.dma_start(rt_sb[:], rT[:])
            lt_sb = const_pool.tile([LORA_DIM, O], bf)
            nc.sync.dma_start(lt_sb[:], lT[:])

            # resident raw weight: [128 kpart, oc, kt, o'] bf16
            wsb = const_pool.tile([128, NOC, NKT, OCW], bf)
            xt_tiles = {}
            xr_tiles = {}

            def load_x(tb):
                xt = xt_pool.tile([128, NKT, TB], bf, tag="xt")
                nc.sync.dma_start(xt[:], xT[tb])
                xt_tiles[tb] = xt

            # input DMA order: x tb0 -> W oc0..3 -> bias (all ahead of any
            # output DMA in the queue); one 4 MiB DMA per W o-chunk
            load_x(0)
            for oc in range(NOC):
                nc.sync.dma_start(wsb[:, oc], wT[:, oc])
            bias_sb = const_pool.tile([128, O], f32)
            nc.sync.dma_start(bias_sb[:], biasr[:])

            # PE warm-up: dependency-free matmuls on garbage SBUF run while
            # the first x/W loads are in flight, so the HAM clock gate is at
            # 8/8 (2.4 GHz) when real matmuls start. Results are never read.
            warm = const_pool.tile([128, OCW], bf)
            nc.any.memset(warm[:], 0.0)

            def warmup(n):
                for _ in range(n):
                    pw = psum_w_pool.tile([128, OCW], f32, tag="pw")
                    nc.tensor.matmul(
                        pw[:], warm[:, :128], warm[:], start=True, stop=True
                    )

            def xr_stage(tb):
                # xr[j, t'] = sum_k R[j, k] x[tb*TB + t', k]: one full-K
                # accumulation group, then a copy to SBUF bf16 for use as
                # the joint matmul's stationary operand
                xt = xt_tiles[tb]
                pr = psum_r_pool.tile([LORA_DIM, TB], f32, tag="pr")
                for kt in range(NKT):
                    nc.tensor.matmul(
                        pr[:],
                        rt_sb[:, kt, :],
                        xt[:, kt, :],
                        start=(kt == 0),
                        stop=(kt == NKT - 1),
                    )
                xr = xr_pool.tile([LORA_DIM, TB], bf, tag="xr")
                nc.vector.tensor_copy(xr[:], pr[:])
                xr_tiles[tb] = xr

            def group(tb, tt, oc):
                xt = xt_tiles[tb]
                ps = psum_pool.tile([128, OCW], f32, tag="ps")
                for kt in range(NKT):
                    nc.tensor.matmul(
                        ps[:],
                        xt[:, kt, ts(tt, 128)],
                        wsb[:, oc, kt, :],
                        start=(kt == 0),
                        stop=False,
                    )
                # LoRA term joins the same accumulation group
                nc.tensor.matmul(
                    ps[:],
                    xr_tiles[tb][:, ts(tt, 128)],
                    lt_sb[:, ts(oc, OCW)],
                    start=False,
                    stop=True,
                )
                # bias-add rides the psum->SBUF copy
                ob = out_pool.tile([128, OCW], f32, tag="ob")
                nc.vector.tensor_add(ob[:], ps[:], bias_sb[:, ts(oc, OCW)])
                nc.sync.dma_start(
                    out[ts(tb * NTT_B + tt, 128), ts(oc, OCW)], ob[:]
                )

            warmup(N_WARMUP_PRE)   # covers the x tb0 DMA window
            xr_stage(0)
            warmup(N_WARMUP_POST)  # covers the rest of the W oc0 DMA window

            for tb in range(NTB):
                if tb + 1 < NTB:
                    load_x(tb + 1)  # prefetch ahead of this tb's out DMAs
                if tb > 0:
                    xr_stage(tb)
                for oc in range(NOC):
                    for tt in range(NTT_B):
                        group(tb, tt, oc)
    return nc


def kernel(**inputs: np.ndarray) -> np.ndarray:
    global LAST_EXEC_TIME_NS, LAST_RESULT
    import ml_dtypes

    bf16 = ml_dtypes.bfloat16

    x = np.asarray(inputs["x"], dtype=np.float32)
    weight = np.asarray(inputs["weight"], dtype=np.float32)
    bias = np.asarray(inputs["bias"], dtype=np.float32)
    lora_left = np.asarray(inputs["lora_left"], dtype=np.float32)
    lora_right = np.asarray(inputs["lora_right"], dtype=np.float32)

    # host-side shard + layout prep (tiled to match SBUF tile order)
    # xT[tb, p, kt, t'] = x[b][tb*TB + t', kt*128 + p]
    xT_shards = [
        np.ascontiguousarray(
            x[b].T.reshape(NKT, 128, NTB, TB).transpose(2, 1, 0, 3)
        ).astype(bf16)
        for b in range(B)
    ]
    # wT[p, oc, kt, o'] = weight[oh*O + oc*OCW + o', kt*128 + p]
    wT_halves = [
        np.ascontiguousarray(
            weight[oh * O : (oh + 1) * O, :].T
            .reshape(NKT, 128, NOC, OCW)
            .transpose(1, 2, 0, 3)
        ).astype(bf16)
        for oh in range(2)
    ]
    # rT[p, kt, j] = lora_right[j, kt*128 + p]
    rT = np.ascontiguousarray(
        lora_right.T.reshape(NKT, 128, LORA_DIM).transpose(1, 0, 2)
    ).astype(bf16)
    lT_halves = [
        np.ascontiguousarray(LORA_SCALE * lora_left[oh * O : (oh + 1) * O, :].T).astype(
            bf16
        )
        for oh in range(2)
    ]
    bias_halves = [
        np.ascontiguousarray(
            np.broadcast_to(bias[None, oh * O : (oh + 1) * O], (128, O))
        )
        for oh in range(2)
    ]

    in_maps = []
    for i in range(N_CORES):
        b, oh = i % B, i // B
        in_maps.append(
            {
                "xT": xT_shards[b],
                "wT": wT_halves[oh],
                "rT": rT,
                "lT": lT_halves[oh],
                "biasr": bias_halves[oh],
            }
        )

    nc = _build_nc()
    res = run_bass_kernel_spmd(
        nc, in_maps, core_ids=list(range(N_CORES)), trace=TRACE
    )
    LAST_EXEC_TIME_NS = res.exec_time_ns
    LAST_RESULT = res

    out = np.empty((B, S, D_OUT), dtype=np.float32)
    for i in range(N_CORES):
        b, oh = i % B, i // B
        out[b, :, oh * O : (oh + 1) * O] = res.results[i]["out"]
    return out


# revision 16
# speedup vs baseline: 1.1942x; 1.0066x over previous
"""Trainium2 Bass kernel for nn_LoRALinear (out = x @ (W + s*L@R)^T + bias).

Full shapes: x [4, 2048, 4096], weight [4096, 4096], bias [4096],
lora_left [4096, 16], lora_right [16, 4096], out [4, 2048, 4096].

Sharding (8 cores, 2D): tokens split 4 ways (the batch dim) x d_out split
2 ways. Core i handles batch b = i % 4 and output half oh = i // 4, i.e. a
[2048, 2048] output block with the full K = 4096 contraction.

Design (v3): single-pass full-K PSUM accumulation with resident raw W.
W stays resident in SBUF in bf16 (128 KB/partition), x streams through
in bf16 (16 KB/partition per 256-token block, double buffered), and each
[128 token x 512 out] PSUM tile accumulates all 32 k-tiles in one group
-- no DRAM partial round-trip (the v1 kernel's kh split cost 64 MiB of
extra HBM traffic and ~100 us). bf16 operands also keep the PE at its
1 cycle/row peak where the v1 f32r kernel measured a 227 ns steady-state
cadence per 512-row matmul (LDWEIGHTS of an fp32 stationary is ~4x the
bf16 cost and was not fully hidden).

LoRA path: per token block, xr = R @ x^T is one 32-matmul PSUM group
([16, 256], full-K accumulation) copied to SBUF bf16; each main PSUM
group then takes a 33rd matmul (xr^T-slice x sL^T-chunk) before stop.
A v2 variant instead pre-merged s*L@R into the resident W with 128
in-place vector adds at startup; that backlogged the vector engine for
~90 us, and the resulting PSUM-reuse stalls on the PE held the HAM
clock gate at 4/8 for ~120 us (569 us total). Keeping the LoRA term on
the PE (+14 us) and the vector engine nearly idle is the faster trade.

Per-core totals: 2048 main + 64 joint + 256 xr matmuls (~478 us of PE
at 2.4 GHz) + ~51 MiB of DMA (~150 us, 3x under the PE time). Bias
rides the PSUM->SBUF copy as a vector add against a
128-partition-replicated bias tile.

DMA ordering: all input DMAs (x block 0, W o-chunks, bias) are enqueued
before any output DMA, and each x block is prefetched one iteration
ahead of use, so in-order queue processing never stalls an input load
behind an output store's semaphore wait. PE warm-up matmuls bracket the
xr stage so the PE has no >700 ns idle gap before steady state (idle
gaps drop the HAM clock gate, which takes ~10 us of busy to re-ramp).
"""

import os
import sys

import numpy as np

for _p in ("/root/.axon_site/_ro/trn_rl_repo", "/opt/trn_rl_repo"):
    if _p not in sys.path and os.path.isdir(_p):
        sys.path.append(_p)

import bass_rust
import concourse.bass as bass
import concourse.mybir as mybir
import concourse.tile as tile
from concourse.bass import ts
from concourse.bass_utils import run_bass_kernel_spmd
from concourse.vector_clock import ScopedClock, VectorClock

# ---- problem constants (hardcoded per contract) ----
B, S, D_IN, D_OUT, LORA_DIM = 4, 2048, 4096, 4096, 16
LORA_SCALE = 32.0 / LORA_DIM
N_CORES = 8
T = 2048          # tokens per core (= one batch element)
O = 2048          # d_out per core (half)
K = D_IN          # contraction
NKT = K // 128    # 32 k-tiles
TB = 256          # token block streamed per x DMA
NTB = T // TB     # 8 token blocks
NTT_B = TB // 128  # 2 token tiles per block
OCW = 512         # o-chunk width (one PSUM bank)
NOC = O // OCW    # 4 o-chunks
N_WARMUP_PRE = 2    # PE warmups before the first xr stage (x block 0 lands early)
N_WARMUP_POST = 20  # PE warmups between xr and the first main group (W oc0 window)

# Set by kernel() after a traced run (test.py reads it).
LAST_EXEC_TIME_NS = None
LAST_RESULT = None
TRACE = False
COMPUTE = "bf16"


class SplitDrainTileContext(tile.TileContext):
    """TileContext that splits multi-wait instructions for this walrus build.

    This walrus rejects instructions carrying >2 sync waits ("Too many sync
    wait commands"). Engine queues are in-order, so an instruction's waits
    can equivalently ride same-engine NOPs inserted just before it; we cap
    every instruction at one wait. Same treatment for the exit Drain.
    """

    _splitw_counter = 0

    def _split_excess_waits(self, ordered):
        for bb_name, insts in ordered.items():
            new_list = []
            changed = False
            for inst in insts:
                si = getattr(inst, "sync_info", None)
                eng = getattr(inst, "engine", mybir.EngineType.Unassigned)
                waits = list(si.on_wait) if si is not None and si.on_wait else []
                if len(waits) > 1 and eng != mybir.EngineType.Unassigned:
                    # keep register-valued waits (if any) on the original
                    movable = [w for w in waits if w.wait_reg is None]
                    pinned = [w for w in waits if w.wait_reg is not None]
                    keep = pinned + movable[-1:] if not pinned else pinned
                    move = movable[:-1] if not pinned else movable
                    for w in move:
                        SplitDrainTileContext._splitw_counter += 1
                        nop = bass_rust.InstNoOp(
                            name=f"tile_splitw_{SplitDrainTileContext._splitw_counter}",
                            ins=[],
                            outs=[],
                        )
                        nop.engine = eng
                        nop.bass_nofuse = True
                        nop.sync_info = bass_rust.SyncInfo(
                            on_wait=[w], on_update=[]
                        )
                        new_list.append(nop)
                    inst.sync_info = bass_rust.SyncInfo(
                        on_wait=keep, on_update=list(si.on_update)
                    )
                    changed = True
                new_list.append(inst)
            if changed:
                insts[:] = new_list

    def _lower_ordered_insts(self, ordered):
        self._split_excess_waits(ordered)
        return super()._lower_ordered_insts(ordered)

    def _drain_and_barrier(self, tick_clock, wait_clock):
        g = tick_clock.global_clock
        for proc in range(len(g)):
            t = g[proc]
            if t <= 0:
                continue
            v = VectorClock()
            v.require_at_least(proc, t)
            nop = self.nc.sync.nop(nofuse=True)
            wait_clock.add_sem_waits(nop.ins, ScopedClock({None: v}))
        drain_inst = self.nc.sync.drain()
        wait_clock.add_sem_waits(
            drain_inst.ins, ScopedClock({None: g}), ScopedClock({None: g})
        )
        self.nc.all_engine_barrier()
        assert self.sems is not None
        popped = self.nc._tile_sem_poison_stack.pop()
        assert popped is self._sem_poison
        self.nc.clear_and_free_semaphores(list(self.sems.allocated().values()))
        self.nc.all_engine_barrier()


def _build_nc() -> bass.Bass:
    f32 = mybir.dt.float32
    bf = mybir.dt.bfloat16

    nc = bass.Bass("TRN2", target_bir_lowering=False, debug=False)
    # host-pre-tiled layouts: each SBUF tile's per-partition bytes are one
    # contiguous DRAM run (max-size DMA descriptors)
    xT = nc.declare_dram_parameter("xT", [NTB, 128, NKT, TB], bf, isOutput=False)
    wT = nc.declare_dram_parameter("wT", [128, NOC, NKT, OCW], bf, isOutput=False)
    rT = nc.declare_dram_parameter("rT", [128, NKT, LORA_DIM], bf, isOutput=False)
    lT = nc.declare_dram_parameter("lT", [LORA_DIM, O], bf, isOutput=False)
    biasr = nc.declare_dram_parameter("biasr", [128, O], f32, isOutput=False)
    out = nc.declare_dram_parameter("out", [T, O], f32, isOutput=True)

    with SplitDrainTileContext(nc) as tc:
        with (
            tc.tile_pool(name="consts", bufs=1) as const_pool,
            tc.tile_pool(name="xt", bufs=2) as xt_pool,
            tc.tile_pool(name="xr", bufs=2) as xr_pool,
            tc.tile_pool(name="outsb", bufs=3) as out_pool,
            tc.tile_pool(name="psum", bufs=5, space="PSUM") as psum_pool,
            tc.tile_pool(name="psum_w", bufs=2, space="PSUM") as psum_w_pool,
            tc.tile_pool(name="psum_r", bufs=1, space="PSUM") as psum_r_pool,
        ):
            # small constants first: R^T, s*L^T (the LoRA matmul operands)
            rt_sb = const_pool.tile([128, NKT, LORA_DIM], bf)
            nc.sync.dma_start(rt_sb[:], rT[:])
            lt_sb = const_pool.tile([LORA_DIM, O], bf)
            nc.sync.dma_start(lt_sb[:], lT[:])

            # resident raw weight: [128 kpart, oc, kt, o'] bf16
            wsb = const_pool.tile([128, NOC, NKT, OCW], bf)
            xt_tiles = {}
            xr_tiles = {}

            def load_x(tb):
                xt = xt_pool.tile([128, NKT, TB], bf, tag="xt")
                nc.sync.dma_start(xt[:], xT[tb])
                xt_tiles[tb] = xt

            # input DMA order: x tb0 -> W oc0 -> bias -> W oc1..3 (all ahead
            # of any output DMA in the queue); one 4 MiB DMA per W o-chunk.
            # bias must land before the first group's bias-add (~t+26us) or
            # the psum-drain path backs up into the PE.
            load_x(0)
            nc.sync.dma_start(wsb[:, 0], wT[:, 0])
            bias_sb = const_pool.tile([128, O], f32)
            nc.sync.dma_start(bias_sb[:], biasr[:])
            for oc in range(1, NOC):
                nc.sync.dma_start(wsb[:, oc], wT[:, oc])

            # PE warm-up: dependency-free matmuls on garbage SBUF run while
            # the first x/W loads are in flight, so the HAM clock gate is at
            # 8/8 (2.4 GHz) when real matmuls start. Results are never read.
            warm = const_pool.tile([128, OCW], bf)
            nc.any.memset(warm[:], 0.0)

            def warmup(n):
                for _ in range(n):
                    pw = psum_w_pool.tile([128, OCW], f32, tag="pw")
                    nc.tensor.matmul(
                        pw[:], warm[:, :128], warm[:], start=True, stop=True
                    )

            def xr_stage(tb):
                # xr[j, t'] = sum_k R[j, k] x[tb*TB + t', k]: one full-K
                # accumulation group, then a copy to SBUF bf16 for use as
                # the joint matmul's stationary operand
                xt = xt_tiles[tb]
                pr = psum_r_pool.tile([LORA_DIM, TB], f32, tag="pr")
                for kt in range(NKT):
                    nc.tensor.matmul(
                        pr[:],
                        rt_sb[:, kt, :],
                        xt[:, kt, :],
                        start=(kt == 0),
                        stop=(kt == NKT - 1),
                    )
                xr = xr_pool.tile([LORA_DIM, TB], bf, tag="xr")
                nc.vector.tensor_copy(xr[:], pr[:])
                xr_tiles[tb] = xr

            def group(tb, tt, oc):
                xt = xt_tiles[tb]
                ps = psum_pool.tile([128, OCW], f32, tag="ps")
                for kt in range(NKT):
                    nc.tensor.matmul(
                        ps[:],
                        xt[:, kt, ts(tt, 128)],
                        wsb[:, oc, kt, :],
                        start=(kt == 0),
                        stop=False,
                    )
                # LoRA term joins the same accumulation group
                nc.tensor.matmul(
                    ps[:],
                    xr_tiles[tb][:, ts(tt, 128)],
                    lt_sb[:, ts(oc, OCW)],
                    start=False,
                    stop=True,
                )
                # bias-add rides the psum->SBUF copy
                ob = out_pool.tile([128, OCW], f32, tag="ob")
                nc.vector.tensor_add(ob[:], ps[:], bias_sb[:, ts(oc, OCW)])
                nc.sync.dma_start(
                    out[ts(tb * NTT_B + tt, 128), ts(oc, OCW)], ob[:]
                )

            warmup(N_WARMUP_PRE)   # covers the x tb0 DMA window
            xr_stage(0)
            warmup(N_WARMUP_POST)  # covers the rest of the W oc0 DMA window

            for tb in range(NTB):
                if tb + 1 < NTB:
                    load_x(tb + 1)  # prefetch ahead of this tb's out DMAs
                if tb > 0:
                    xr_stage(tb)
                for oc in range(NOC):
                    for tt in range(NTT_B):
                        group(tb, tt, oc)
    return nc


def kernel(**inputs: np.ndarray) -> np.ndarray:
    global LAST_EXEC_TIME_NS, LAST_RESULT
    import ml_dtypes

    bf16 = ml_dtypes.bfloat16

    x = np.asarray(inputs["x"], dtype=np.float32)
    weight = np.asarray(inputs["weight"], dtype=np.float32)
    bias = np.asarray(inputs["bias"], dtype=np.float32)
    lora_left = np.asarray(inputs["lora_left"], dtype=np.float32)
    lora_right = np.asarray(inputs["lora_right"], dtype=np.float32)

    # host-side shard + layout prep (tiled to match SBUF tile order)
    # xT[tb, p, kt, t'] = x[b][tb*TB + t', kt*128 + p]
    xT_shards = [
        np.ascontiguousarray(
            x[b].T.reshape(NKT, 128, NTB, TB).transpose(2, 1, 0, 3)
        ).astype(bf16)
        for b in range(B)
    ]
    # wT[p, oc, kt, o'] = weight[oh*O + oc*OCW + o', kt*128 + p]
    wT_halves = [
        np.ascontiguousarray(
            weight[oh * O : (oh + 1) * O, :].T
            .reshape(NKT, 128, NOC, OCW)
            .transpose(1, 2, 0, 3)
        ).astype(bf16)
        for oh in range(2)
    ]
    # rT[p, kt, j] = lora_right[j, kt*128 + p]
    rT = np.ascontiguousarray(
        lora_right.T.reshape(NKT, 128, LORA_DIM).transpose(1, 0, 2)
    ).astype(bf16)
    lT_halves = [
        np.ascontiguousarray(LORA_SCALE * lora_left[oh * O : (oh + 1) * O, :].T).astype(
            bf16
        )
        for oh in range(2)
    ]
    bias_halves = [
        np.ascontiguousarray(
            np.broadcast_to(bias[None, oh * O : (oh + 1) * O], (128, O))
        )
        for oh in range(2)
    ]

    in_maps = []
    for i in range(N_CORES):
        b, oh = i % B, i // B
        in_maps.append(
            {
                "xT": xT_shards[b],
                "wT": wT_halves[oh],
                "rT": rT,
                "lT": lT_halves[oh],
                "biasr": bias_halves[oh],
            }
        )

    nc = _build_nc()
    res = run_bass_kernel_spmd(
        nc, in_maps, core_ids=list(range(N_CORES)), trace=TRACE
    )
    LAST_EXEC_TIME_NS = res.exec_time_ns
    LAST_RESULT = res

    out = np.empty((B, S, D_OUT), dtype=np.float32)
    for i in range(N_CORES):
        b, oh = i % B, i // B
        out[b, :, oh * O : (oh + 1) * O] = res.results[i]["out"]
    return out
